# revision 11
# baseline (speedup 1.0000x reference)
"""Trainium2 Bass kernel for nn_CrossAttention (RMSNorm + SwiGLU FFN + residual +
per-head single-query cross-attention), data-parallel over batch across 8 cores.

Hardcoded problem shapes: B=256, T=200, N=8, D=64, MODEL=512, FFN_H=2048.
Sharding: batch B split 8 ways (32 per core). Each core runs the full fused
pipeline in a feature-major layout (model dim on partitions, tokens on the
free axis, 2 batch rows = 400 tokens per block) with float32r matmuls.
"""
import sys

sys.path.insert(0, "/opt/trn_rl_repo")

import numpy as np  # noqa: E402

B, T, NH, D = 256, 200, 8, 64
MODEL = NH * D            # 512
FFN_H = 1024 * NH // 4    # 2048
EPS = 1.1920929e-07
NCORES = 8
BS = B // NCORES          # 32 batch rows per core
NBLK = BS // 2            # 16 blocks of 2 batch rows
TT = 2 * T                # 400 tokens per block
NEG = -1.0e30

_RUNNERS = {}


def _build_program(repeat=1):
    import concourse.tile as tile
    from concourse import mybir, bacc
    from concourse.masks import make_identity
    import concourse.bass as bass

    f32 = mybir.dt.float32
    f32r = mybir.dt.float32r
    u8 = mybir.dt.uint8
    Alu = mybir.AluOpType
    Act = mybir.ActivationFunctionType

    nc = bacc.Bacc(trn_type="TRN2", target_bir_lowering=False, debug=False,
                   num_devices=NCORES)

    seq_d = nc.dram_tensor("seq", [BS, T, MODEL], f32, kind="ExternalInput")
    q_d = nc.dram_tensor("q", [BS, MODEL], f32, kind="ExternalInput")
    mask_d = nc.dram_tensor("mask", [BS, T], u8, kind="ExternalInput")
    rms_d = nc.dram_tensor("rms_w", [MODEL], f32, kind="ExternalInput")
    w1_d = nc.dram_tensor("w1", [MODEL, FFN_H], f32r, kind="ExternalInput")
    w2_d = nc.dram_tensor("w2", [MODEL, FFN_H], f32r, kind="ExternalInput")
    w3_d = nc.dram_tensor("w3", [FFN_H, MODEL], f32r, kind="ExternalInput")
    wk_d = nc.dram_tensor("w_k", [NH, D, D], f32, kind="ExternalInput")
    wv_d = nc.dram_tensor("w_v", [NH, D, D], f32r, kind="ExternalInput")
    out_d = nc.dram_tensor("out", [BS, MODEL], f32, kind="ExternalOutput")

    with tile.TileContext(nc) as tc:
        with (
            nc.allow_low_precision(reason="float32r tiles are fp32-width storage"),
            tc.tile_pool(name="wpool", bufs=1) as wpool,
            tc.tile_pool(name="const", bufs=1) as const,
            tc.tile_pool(name="tok", bufs=3) as tokp,
            tc.tile_pool(name="big", bufs=1) as big,
            tc.tile_pool(name="work", bufs=2) as work,
            tc.tile_pool(name="attn", bufs=2) as attnp,
            tc.tile_pool(name="psA", bufs=2, space="PSUM") as psA,   # misc: transposes, ssq, rstd_bc
            tc.tile_pool(name="psB", bufs=2, space="PSUM") as psB,   # small: scores + attn bcast
            tc.tile_pool(name="psU", bufs=2, space="PSUM") as psU,   # stage1 u1/u2
            tc.tile_pool(name="psY", bufs=2, space="PSUM") as psY,   # stage2 y + wv out
        ):
            # ---------------- one-time constants / weights ----------------
            w1_sb = wpool.tile([128, 4, FFN_H], f32r, name="w1_sb")
            nc.sync.dma_start(w1_sb[:], w1_d.ap().rearrange("(ko p) f -> p ko f", p=128))
            w2_sb = wpool.tile([128, 4, FFN_H], f32r, name="w2_sb")
            nc.sync.dma_start(w2_sb[:], w2_d.ap().rearrange("(ko p) f -> p ko f", p=128))
            w3_sb = wpool.tile([128, 16, MODEL], f32r, name="w3_sb")
            nc.sync.dma_start(w3_sb[:], w3_d.ap().rearrange("(ko p) m -> p ko m", p=128))
            rms_sb = const.tile([128, 4], f32, name="rms_sb")
            nc.sync.dma_start(rms_sb[:], rms_d.ap().rearrange("(ko p) -> p ko", p=128))

            ident = const.tile([128, 128], f32, name="ident")
            make_identity(nc, ident[:])

            stage = const.tile([128, 4, NH], f32, name="stage")
            nc.vector.memset(stage[:], 1.0)
            ones_col = const.tile([128, 1], f32r, name="ones_col")
            nc.vector.tensor_copy(ones_col[:], stage[:, 0, 0:1])
            ones_row = const.tile([1, 128], f32r, name="ones_row")
            nc.vector.tensor_copy(ones_row[:], stage[0:1, 0, 0:1].to_broadcast([1, 128]))
            ones_m = const.tile([128, NH], mybir.dt.bfloat16, name="ones_m")
            nc.vector.tensor_copy(ones_m[:], stage[:, 0, :])

            nc.vector.memset(stage[:], 0.0)
            for c in range(4):
                for j in range(2):
                    n = 2 * c + j
                    nc.vector.memset(stage[64 * j:64 * (j + 1), c, n:n + 1], 1.0)
            ones_blk = const.tile([128, 4, NH], f32r, name="ones_blk")
            nc.vector.tensor_copy(ones_blk[:], stage[:])
            # sel8[n, c, p] = ones_blk[p, c, n]: via PE transpose (memset can't
            # target arbitrary partition bases)
            ident_r = const.tile([128, 128], f32r, name="ident_r")
            nc.vector.tensor_copy(ident_r[:], ident[:])
            sel8 = const.tile([NH, 4, 128], f32r, name="sel8")
            for c in range(4):
                p_sel = psA.tile([NH, 128], f32r, name="p_sel", tag="ptrans")
                nc.tensor.transpose(p_sel[:], ones_blk[:, c, :], ident_r[:])
                nc.vector.tensor_copy(sel8[:, c, :], p_sel[:])

            eps_t = const.tile([1, 1], f32, name="eps_t")
            nc.vector.memset(eps_t[:], EPS)

            # mask -> additive -1e30 rows, at partitions {0, 64} (16 rows each)
            mu8 = const.tile([128, 16, T], u8, name="mu8")
            for p in range(2):
                src = bass.AP(tensor=mask_d, offset=p * 16 * T,
                              ap=[[0, 1], [T, 16], [1, T]])
                nc.sync.dma_start(mu8[64 * p:64 * p + 1, :, :], src)
            maskadd = const.tile([128, 16, T], mybir.dt.bfloat16, name="maskadd")
            nc.vector.tensor_scalar(maskadd[:], mu8[:], 1.0, -NEG,
                                    op0=Alu.subtract, op1=Alu.mult)

            # q: load token-major, transpose per head, fold w_k -> qkT
            q_sb = const.tile([BS, MODEL], f32, name="q_sb")
            nc.sync.dma_start(q_sb[:], q_d.ap())
            wk_sb = const.tile([64, NH, D], f32, name="wk_sb")
            nc.sync.dma_start(wk_sb[:], wk_d.ap().rearrange("n d e -> d n e"))
            # block-diag(w_v[2c], w_v[2c+1]) per chunk: one K=128 matmul per chunk
            wv_stage = const.tile([128, 4, 128], f32, name="wv_stage")
            nc.vector.memset(wv_stage[:], 0.0)
            wv_sb = const.tile([128, 4, 128], f32r, name="wv_sb")
            nc.vector.tensor_copy(wv_sb[:], wv_stage[:])
            for n in range(NH):
                j, c = n % 2, n // 2
                nc.sync.dma_start(wv_sb[64 * j:64 * (j + 1), c, 64 * j:64 * (j + 1)], wv_d.ap()[n])

            qT_sb = const.tile([64, NH, BS], f32r, name="qT_sb")
            wkT_sb = const.tile([64, NH, D], f32r, name="wkT_sb")
            qkT_sb = const.tile([128, 4, BS], f32, name="qkT_sb")
            for n in range(NH):
                p_qt = psA.tile([64, BS], f32, name="p_qt", tag="ptrans")
                nc.tensor.transpose(p_qt[:], q_sb[:, 64 * n:64 * (n + 1)], ident[:BS, :BS])
                nc.vector.tensor_copy(qT_sb[:, n, :], p_qt[:])
                p_wkt = psA.tile([64, D], f32, name="p_wkt", tag="ptrans")
                nc.tensor.transpose(p_wkt[:], wk_sb[:, n, :], ident[:64, :64])
                nc.vector.tensor_copy(wkT_sb[:, n, :], p_wkt[:])
            for n in range(NH):
                j, c = n % 2, n // 2
                p_qk = psA.tile([64, BS], f32, name="p_qk", tag="ptrans")
                nc.tensor.matmul(p_qk[:], wkT_sb[:, n, :], qT_sb[:, n, :],
                                 start=True, stop=True)
                nc.scalar.mul(qkT_sb[64 * j:64 * (j + 1), c, :], p_qk[:], D ** -0.5)

            ctxT = big.tile([128, 4, BS], f32, name="ctxT")

            # ---------------- main pipeline ----------------
            for rep in range(repeat):
                for bi in range(NBLK):
                    seqT = big.tile([128, 4, TT], f32, name="seqT", tag="seqT")
                    # load + transpose 2 batch rows
                    for bb2 in range(2):
                        b = 2 * bi + bb2
                        off = bb2 * T
                        tok0 = tokp.tile([128, MODEL], f32, name="tok0", tag="tok0")
                        nc.sync.dma_start(tok0[:], seq_d.ap()[b, 0:128, :])
                        tok1 = tokp.tile([72, MODEL], f32, name="tok1", tag="tok1")
                        nc.sync.dma_start(tok1[:], seq_d.ap()[b, 128:T, :])
                        for c in range(4):
                            pt0 = psA.tile([128, 128], f32, name="pt0", tag="ptrans")
                            nc.tensor.transpose(pt0[:], tok0[:, 128 * c:128 * (c + 1)], ident[:])
                            nc.scalar.copy(seqT[:, c, off:off + 128], pt0[:])
                            pt1 = psA.tile([128, 72], f32, name="pt1", tag="ptrans")
                            nc.tensor.transpose(pt1[:], tok1[:, 128 * c:128 * (c + 1)], ident[:72, :72])
                            nc.scalar.copy(seqT[:, c, off + 128:off + 200], pt1[:])

                    # RMS norm (feature-major): ssq over partitions via ones-matmul
                    p_ssq = psA.tile([1, TT], f32, name="p_ssq", tag="ptrans")
                    for c in range(4):
                        sq = work.tile([128, TT], f32r, name="sq", tag="sq")
                        nc.vector.tensor_mul(sq[:], seqT[:, c, :], seqT[:, c, :])
                        nc.tensor.matmul(p_ssq[:], ones_col[:], sq[:],
                                         start=(c == 0), stop=(c == 3))
                    sqv = work.tile([1, TT], f32, name="sqv", tag="sqv")
                    nc.scalar.activation(sqv[:], p_ssq[:], Act.Sqrt,
                                         bias=eps_t[:], scale=1.0 / MODEL)
                    rstd = work.tile([1, TT], f32r, name="rstd", tag="rstd")
                    nc.vector.reciprocal(rstd[:], sqv[:])
                    p_bc = psA.tile([128, TT], f32, name="p_bc", tag="ptrans")
                    nc.tensor.matmul(p_bc[:], ones_row[:], rstd[:], start=True, stop=True)
                    snT = big.tile([128, 4, TT], f32r, name="snT", tag="snT")
                    for c in range(4):
                        nc.vector.scalar_tensor_tensor(
                            snT[:, c, :], seqT[:, c, :], rms_sb[:, c:c + 1], p_bc[:],
                            op0=Alu.mult, op1=Alu.mult)

                    # stage 1: gT[f,t] = silu(w1.T@snT) * (w2.T@snT)
                    gT = big.tile([128, 16, TT], f32r, name="gT", tag="gT")
                    for f in range(16):
                        fs = slice(128 * f, 128 * (f + 1))
                        pu1 = psU.tile([128, TT], f32, name="pu1", tag="u")
                        for c in range(4):
                            nc.tensor.matmul(pu1[:], w1_sb[:, c, fs], snT[:, c, :],
                                             start=(c == 0), stop=(c == 3))
                        pu2 = psU.tile([128, TT], f32, name="pu2", tag="u")
                        for c in range(4):
                            nc.tensor.matmul(pu2[:], w2_sb[:, c, fs], snT[:, c, :],
                                             start=(c == 0), stop=(c == 3))
                        sil = work.tile([128, TT], f32, name="sil", tag="sil")
                        nc.scalar.activation(sil[:], pu1[:], Act.Silu)
                        nc.vector.tensor_mul(gT[:, f, :], sil[:], pu2[:])

                    # stage 2: hT = w3.T@gT + seqT   (residual)
                    hT = big.tile([128, 4, TT], f32r, name="hT", tag="hT")
                    for m in range(4):
                        ms = slice(128 * m, 128 * (m + 1))
                        py = psY.tile([128, TT], f32, name="py", tag="yv")
                        for kf in range(16):
                            nc.tensor.matmul(py[:], w3_sb[:, kf, ms], gT[:, kf, :],
                                             start=(kf == 0), stop=(kf == 15))
                        nc.vector.tensor_add(hT[:, m, :], py[:], seqT[:, m, :])

                    # per-head value projection vT[e,t] (2 heads per psum tile)
                    vT = big.tile([128, 4, TT], f32, name="vT", tag="vT")
                    for c in range(4):
                        pv = psY.tile([128, TT], f32, name="pv", tag="yv")
                        nc.tensor.matmul(pv[:], wv_sb[:, c, :], hT[:, c, :],
                                         start=True, stop=True)
                        nc.scalar.copy(vT[:, c, :], pv[:])

                    # attention per batch row
                    for bb2 in range(2):
                        b = 2 * bi + bb2
                        off = bb2 * T
                        ps = psB.tile([NH, T], f32, name="ps", tag="sc")
                        for c in range(4):
                            tmp = work.tile([128, T], f32r, name="tmp", tag="tmp")
                            nc.vector.tensor_scalar_mul(tmp[:], hT[:, c, off:off + T],
                                                        qkT_sb[:, c, b:b + 1])
                            nc.tensor.matmul(ps[:], ones_blk[:, c, :], tmp[:],
                                             start=(c == 0), stop=False)
                        mp, mj = b // 16, b % 16
                        nc.tensor.matmul(ps[:], ones_m[64 * mp:64 * mp + 1, :],
                                         maskadd[64 * mp:64 * mp + 1, mj, :],
                                         start=False, stop=True)
                        negmx = attnp.tile([NH, 1], f32, name="negmx", tag="negmx")
                        nc.vector.tensor_reduce(negmx[:], ps[:], axis=mybir.AxisListType.X,
                                                op=Alu.max, negate=True)
                        attn_e = attnp.tile([NH, T], f32, name="attn_e", tag="attn_e")
                        den = attnp.tile([NH, 1], f32, name="den", tag="den")
                        nc.scalar.activation(attn_e[:], ps[:], Act.Exp,
                                             bias=negmx[:], scale=1.0, accum_out=den[:])
                        rden = attnp.tile([NH, 1], f32, name="rden", tag="rden")
                        nc.vector.reciprocal(rden[:], den[:])
                        attn_n = attnp.tile([NH, T], f32r, name="attn_n", tag="attn_n")
                        nc.vector.tensor_scalar_mul(attn_n[:], attn_e[:], rden[:])
                        for c in range(4):
                            pbc2 = psB.tile([128, T], f32, name="pbc2", tag="sc")
                            nc.tensor.matmul(pbc2[:], sel8[:, c, :], attn_n[:],
                                             start=True, stop=True)
                            scr = work.tile([128, T], f32, name="scr", tag="scr")
                            nc.vector.scalar_tensor_tensor(
                                scr[:], vT[:, c, off:off + T], 1.0, pbc2[:],
                                op0=Alu.mult, op1=Alu.mult,
                                accum_out=ctxT[:, c, b:b + 1])

            # ---------------- output: transpose back + residual q ----------------
            ctx_tok = const.tile([BS, 4, 128], f32, name="ctx_tok")
            for c in range(4):
                pt = psA.tile([BS, 128], f32, name="pt", tag="ptrans")
                nc.tensor.transpose(pt[:], ctxT[:, c, :], ident[:])
                nc.vector.scalar_tensor_tensor(
                    ctx_tok[:, c, :], pt[:], 1.0, q_sb[:, 128 * c:128 * (c + 1)],
                    op0=Alu.mult, op1=Alu.add)
            nc.sync.dma_start(out_d.ap(), ctx_tok[:].rearrange("b c p -> b (c p)"))

    nc.compile()
    return nc


def _make_runner(nc, n_cores=NCORES):
    import jax
    from jax.sharding import Mesh, PartitionSpec, NamedSharding
    from jax.experimental.shard_map import shard_map
    from concourse import mybir as _mybir
    from concourse import bass2jax

    bass2jax.install_neuronx_cc_hook()
    partition_name = nc.partition_id_tensor.name if nc.partition_id_tensor else None
    in_names, out_names, out_avals, zero_outs = [], [], [], []
    for alloc in nc.m.functions[0].allocations:
        if not isinstance(alloc, _mybir.MemoryLocationSet):
            continue
        name = alloc.memorylocations[0].name
        if alloc.kind == "ExternalInput":
            if name != partition_name:
                in_names.append(name)
        elif alloc.kind == "ExternalOutput":
            out_names.append(name)
            shape = tuple(alloc.tensor_shape)
            dtype = _mybir.dt.np(alloc.dtype)
            out_avals.append(jax.core.ShapedArray(shape, dtype))
            zero_outs.append(np.zeros(shape, dtype))
    n_params = len(in_names)
    all_in_names = list(in_names) + list(out_names)
    if partition_name is not None:
        all_in_names.append(partition_name)

    def _body(*args):
        operands = list(args)
        if partition_name is not None:
            operands.append(bass2jax.partition_id_tensor())
        outs = bass2jax._bass_exec_p.bind(
            *operands,
            out_avals=tuple(out_avals),
            in_names=tuple(all_in_names),
            out_names=tuple(out_names),
            lowering_input_output_aliases=(),
            sim_require_finite=True,
            sim_require_nnan=True,
            nc=nc,
        )
        return tuple(outs)

    devices = jax.devices()[:n_cores]
    mesh = Mesh(np.asarray(devices), ("core",))
    n_outs = len(out_avals)
    in_specs = (PartitionSpec("core"),) * (n_params + n_outs)
    out_specs = (PartitionSpec("core"),) * len(out_names)
    sharded = jax.jit(
        shard_map(_body, mesh=mesh, in_specs=in_specs, out_specs=out_specs,
                  check_rep=False),
        keep_unused=True)
    sharding = NamedSharding(mesh, PartitionSpec("core"))

    def run(in_maps, timing_iters=0):
        import time
        per_core = [[np.asarray(m[name]) for name in in_names] for m in in_maps]
        concat_in = [np.concatenate([per_core[c][i] for c in range(n_cores)], axis=0)
                     for i in range(n_params)]
        concat_zeros = [np.zeros((n_cores * z.shape[0], *z.shape[1:]), z.dtype)
                        for z in zero_outs]
        dev_in = [jax.device_put(x, sharding) for x in concat_in]
        dev_zero = [jax.device_put(x, sharding) for x in concat_zeros]
        out = sharded(*dev_in, *dev_zero)
        jax.block_until_ready(out)
        dt = None
        if timing_iters:
            t0 = time.perf_counter()
            for _ in range(timing_iters):
                out = sharded(*dev_in, *dev_zero)
            jax.block_until_ready(out)
            dt = (time.perf_counter() - t0) / timing_iters
        results = [
            {name: np.asarray(out[i]).reshape(n_cores, *out_avals[i].shape)[c]
             for i, name in enumerate(out_names)}
            for c in range(n_cores)
        ]
        return results, dt

    return run


def _get_runner(repeat=1):
    if repeat not in _RUNNERS:
        nc = _build_program(repeat=repeat)
        _RUNNERS[repeat] = _make_runner(nc)
    return _RUNNERS[repeat]


def _in_maps(q, seq, rms_w, w1, w2, w3, w_k, w_v, seq_mask):
    q = np.asarray(q, np.float32).reshape(B, MODEL)
    seq = np.asarray(seq, np.float32)
    mask = np.asarray(seq_mask).astype(np.uint8)
    rms_w = np.asarray(rms_w, np.float32)
    w1 = np.asarray(w1, np.float32)
    w2 = np.asarray(w2, np.float32)
    w3 = np.asarray(w3, np.float32)
    w_k = np.asarray(w_k, np.float32)
    w_v = np.asarray(w_v, np.float32)
    maps = []
    for c in range(NCORES):
        s = slice(c * BS, (c + 1) * BS)
        maps.append({
            "seq": seq[s], "q": q[s], "mask": mask[s], "rms_w": rms_w,
            "w1": w1, "w2": w2, "w3": w3, "w_k": w_k, "w_v": w_v,
        })
    return maps


def kernel(q, seq, rms_w, w1, w2, w3, w_k, w_v, seq_mask, _timing_iters=0, _repeat=1):
    run = _get_runner(_repeat)
    maps = _in_maps(q, seq, rms_w, w1, w2, w3, w_k, w_v, seq_mask)
    results, dt = run(maps, timing_iters=_timing_iters)
    out = np.concatenate([r["out"] for r in results], axis=0).reshape(B, NH, D)
    if _timing_iters:
        return out, dt
    return out


# revision 12
# speedup vs baseline: 1.4336x; 1.4336x over previous
"""Trainium2 Bass kernel for nn_CrossAttention (RMSNorm + SwiGLU FFN + residual +
per-head single-query cross-attention), data-parallel over batch across 8 cores.

Hardcoded problem shapes: B=256, T=200, N=8, D=64, MODEL=512, FFN_H=2048.
Sharding: batch B split 8 ways (32 per core). Each core runs the full fused
pipeline in a feature-major layout (model dim on partitions, tokens on the
free axis, 2 batch rows = 400 tokens per block) with float32r matmuls.
"""
import sys

sys.path.insert(0, "/opt/trn_rl_repo")

import numpy as np  # noqa: E402

B, T, NH, D = 256, 200, 8, 64
MODEL = NH * D            # 512
FFN_H = 1024 * NH // 4    # 2048
EPS = 1.1920929e-07
NCORES = 8
BS = B // NCORES          # 32 batch rows per core
NBLK = BS // 2            # 16 blocks of 2 batch rows
TT = 2 * T                # 400 tokens per block
NEG = -1.0e30

_RUNNERS = {}


def _build_program(repeat=1):
    import concourse.tile as tile
    from concourse import mybir, bacc
    from concourse.masks import make_identity
    import concourse.bass as bass

    f32 = mybir.dt.float32
    f32r = mybir.dt.float32r
    u8 = mybir.dt.uint8
    Alu = mybir.AluOpType
    Act = mybir.ActivationFunctionType

    nc = bacc.Bacc(trn_type="TRN2", target_bir_lowering=False, debug=False,
                   num_devices=NCORES)

    seq_d = nc.dram_tensor("seq", [BS, T, MODEL], f32, kind="ExternalInput")
    q_d = nc.dram_tensor("q", [BS, MODEL], f32, kind="ExternalInput")
    mask_d = nc.dram_tensor("mask", [BS, T], u8, kind="ExternalInput")
    rms_d = nc.dram_tensor("rms_w", [MODEL], f32, kind="ExternalInput")
    w1_d = nc.dram_tensor("w1", [MODEL, FFN_H], f32r, kind="ExternalInput")
    w2_d = nc.dram_tensor("w2", [MODEL, FFN_H], f32r, kind="ExternalInput")
    w3_d = nc.dram_tensor("w3", [FFN_H, MODEL], f32r, kind="ExternalInput")
    wk_d = nc.dram_tensor("w_k", [NH, D, D], f32, kind="ExternalInput")
    wv_d = nc.dram_tensor("w_v", [NH, D, D], f32r, kind="ExternalInput")
    out_d = nc.dram_tensor("out", [BS, MODEL], f32, kind="ExternalOutput")

    with tile.TileContext(nc) as tc:
        with (
            nc.allow_low_precision(reason="float32r tiles are fp32-width storage"),
            tc.tile_pool(name="wpool", bufs=1) as wpool,
            tc.tile_pool(name="const", bufs=1) as const,
            tc.tile_pool(name="tok", bufs=3) as tokp,
            tc.tile_pool(name="big", bufs=1) as big,
            tc.tile_pool(name="work", bufs=2) as work,
            tc.tile_pool(name="attn", bufs=2) as attnp,
            tc.tile_pool(name="psA", bufs=2, space="PSUM") as psA,   # misc: transposes, ssq, rstd_bc
            tc.tile_pool(name="psB", bufs=2, space="PSUM") as psB,   # small: scores + attn bcast
            tc.tile_pool(name="psU", bufs=2, space="PSUM") as psU,   # stage1 u1/u2
            tc.tile_pool(name="psY", bufs=2, space="PSUM") as psY,   # stage2 y + wv out
        ):
            # ---------------- one-time constants / weights ----------------
            w1_sb = wpool.tile([128, 4, FFN_H], f32r, name="w1_sb")
            nc.sync.dma_start(w1_sb[:], w1_d.ap().rearrange("(ko p) f -> p ko f", p=128))
            w2_sb = wpool.tile([128, 4, FFN_H], f32r, name="w2_sb")
            nc.sync.dma_start(w2_sb[:], w2_d.ap().rearrange("(ko p) f -> p ko f", p=128))
            w3_sb = wpool.tile([128, 16, MODEL], f32r, name="w3_sb")
            nc.sync.dma_start(w3_sb[:], w3_d.ap().rearrange("(ko p) m -> p ko m", p=128))
            rms_sb = const.tile([128, 4], f32, name="rms_sb")
            nc.sync.dma_start(rms_sb[:], rms_d.ap().rearrange("(ko p) -> p ko", p=128))

            ident = const.tile([128, 128], f32, name="ident")
            make_identity(nc, ident[:])

            stage = const.tile([128, 4, NH], f32, name="stage")
            nc.vector.memset(stage[:], 1.0)
            ones_col = const.tile([128, 1], f32r, name="ones_col")
            nc.vector.tensor_copy(ones_col[:], stage[:, 0, 0:1])
            ones_row = const.tile([1, 128], f32r, name="ones_row")
            nc.vector.tensor_copy(ones_row[:], stage[0:1, 0, 0:1].to_broadcast([1, 128]))
            ones_m = const.tile([128, NH], mybir.dt.bfloat16, name="ones_m")
            nc.vector.tensor_copy(ones_m[:], stage[:, 0, :])

            nc.vector.memset(stage[:], 0.0)
            for c in range(4):
                for j in range(2):
                    n = 2 * c + j
                    nc.vector.memset(stage[64 * j:64 * (j + 1), c, n:n + 1], 1.0)
            ones_blk = const.tile([128, 4, NH], f32r, name="ones_blk")
            nc.vector.tensor_copy(ones_blk[:], stage[:])
            # sel8[n, c, p] = ones_blk[p, c, n]: via PE transpose (memset can't
            # target arbitrary partition bases)
            ident_r = const.tile([128, 128], f32r, name="ident_r")
            nc.vector.tensor_copy(ident_r[:], ident[:])
            sel8 = const.tile([NH, 4, 128], f32r, name="sel8")
            for c in range(4):
                p_sel = psA.tile([NH, 128], f32r, name="p_sel", tag="ptrans")
                nc.tensor.transpose(p_sel[:], ones_blk[:, c, :], ident_r[:])
                nc.vector.tensor_copy(sel8[:, c, :], p_sel[:])

            eps_t = const.tile([1, 1], f32, name="eps_t")
            nc.vector.memset(eps_t[:], EPS)

            # mask -> additive -1e30 rows, at partitions {0, 64} (16 rows each)
            mu8 = const.tile([128, 16, T], u8, name="mu8")
            for p in range(2):
                src = bass.AP(tensor=mask_d, offset=p * 16 * T,
                              ap=[[0, 1], [T, 16], [1, T]])
                nc.sync.dma_start(mu8[64 * p:64 * p + 1, :, :], src)
            maskadd = const.tile([128, 16, T], mybir.dt.bfloat16, name="maskadd")
            nc.vector.tensor_scalar(maskadd[:], mu8[:], 1.0, -NEG,
                                    op0=Alu.subtract, op1=Alu.mult)

            # q: load token-major, transpose per head, fold w_k -> qkT
            q_sb = const.tile([BS, MODEL], f32, name="q_sb")
            nc.sync.dma_start(q_sb[:], q_d.ap())
            wk_sb = const.tile([64, NH, D], f32, name="wk_sb")
            nc.sync.dma_start(wk_sb[:], wk_d.ap().rearrange("n d e -> d n e"))
            # block-diag(w_v[2c], w_v[2c+1]) per chunk: one K=128 matmul per chunk
            wv_stage = const.tile([128, 4, 128], f32, name="wv_stage")
            nc.vector.memset(wv_stage[:], 0.0)
            wv_sb = const.tile([128, 4, 128], f32r, name="wv_sb")
            nc.vector.tensor_copy(wv_sb[:], wv_stage[:])
            for n in range(NH):
                j, c = n % 2, n // 2
                nc.sync.dma_start(wv_sb[64 * j:64 * (j + 1), c, 64 * j:64 * (j + 1)], wv_d.ap()[n])

            qT_sb = const.tile([64, NH, BS], f32r, name="qT_sb")
            wkT_sb = const.tile([64, NH, D], f32r, name="wkT_sb")
            qkT_sb = const.tile([128, 4, BS], f32, name="qkT_sb")
            for n in range(NH):
                p_qt = psA.tile([64, BS], f32, name="p_qt", tag="ptrans")
                nc.tensor.transpose(p_qt[:], q_sb[:, 64 * n:64 * (n + 1)], ident[:BS, :BS])
                nc.vector.tensor_copy(qT_sb[:, n, :], p_qt[:])
                p_wkt = psA.tile([64, D], f32, name="p_wkt", tag="ptrans")
                nc.tensor.transpose(p_wkt[:], wk_sb[:, n, :], ident[:64, :64])
                nc.vector.tensor_copy(wkT_sb[:, n, :], p_wkt[:])
            for n in range(NH):
                j, c = n % 2, n // 2
                p_qk = psA.tile([64, BS], f32, name="p_qk", tag="ptrans")
                nc.tensor.matmul(p_qk[:], wkT_sb[:, n, :], qT_sb[:, n, :],
                                 start=True, stop=True)
                nc.scalar.mul(qkT_sb[64 * j:64 * (j + 1), c, :], p_qk[:], D ** -0.5)

            ctxT = big.tile([128, 4, BS], f32, name="ctxT")
            # keeps every repetition live under DCE when repeat > 1
            dce_guard = big.tile([128, 4, BS], f32, name="dce_guard")
            nc.vector.memset(dce_guard[:], 0.0)

            # ---------------- main pipeline ----------------
            for rep in range(repeat):
                for bi in range(NBLK):
                    seqT = big.tile([128, 4, TT], f32, name="seqT", tag="seqT")
                    # load + transpose 2 batch rows
                    for bb2 in range(2):
                        b = 2 * bi + bb2
                        off = bb2 * T
                        tok0 = tokp.tile([128, MODEL], f32, name="tok0", tag="tok0")
                        nc.sync.dma_start(tok0[:], seq_d.ap()[b, 0:128, :])
                        tok1 = tokp.tile([72, MODEL], f32, name="tok1", tag="tok1")
                        nc.sync.dma_start(tok1[:], seq_d.ap()[b, 128:T, :])
                        for c in range(4):
                            pt0 = psA.tile([128, 128], f32, name="pt0", tag="ptrans")
                            nc.tensor.transpose(pt0[:], tok0[:, 128 * c:128 * (c + 1)], ident[:])
                            nc.scalar.copy(seqT[:, c, off:off + 128], pt0[:])
                            pt1 = psA.tile([128, 72], f32, name="pt1", tag="ptrans")
                            nc.tensor.transpose(pt1[:], tok1[:, 128 * c:128 * (c + 1)], ident[:72, :72])
                            nc.scalar.copy(seqT[:, c, off + 128:off + 200], pt1[:])

                    # RMS norm (feature-major): ssq over partitions via ones-matmul
                    p_ssq = psA.tile([1, TT], f32, name="p_ssq", tag="ptrans")
                    for c in range(4):
                        sq = work.tile([128, TT], f32r, name="sq", tag="sq")
                        nc.vector.tensor_mul(sq[:], seqT[:, c, :], seqT[:, c, :])
                        nc.tensor.matmul(p_ssq[:], ones_col[:], sq[:],
                                         start=(c == 0), stop=(c == 3))
                    sqv = work.tile([1, TT], f32, name="sqv", tag="sqv")
                    nc.scalar.activation(sqv[:], p_ssq[:], Act.Sqrt,
                                         bias=eps_t[:], scale=1.0 / MODEL)
                    rstd = work.tile([1, TT], f32r, name="rstd", tag="rstd")
                    nc.vector.reciprocal(rstd[:], sqv[:])
                    p_bc = psA.tile([128, TT], f32, name="p_bc", tag="ptrans")
                    nc.tensor.matmul(p_bc[:], ones_row[:], rstd[:], start=True, stop=True)
                    snT = big.tile([128, 4, TT], f32r, name="snT", tag="snT")
                    for c in range(4):
                        nc.vector.scalar_tensor_tensor(
                            snT[:, c, :], seqT[:, c, :], rms_sb[:, c:c + 1], p_bc[:],
                            op0=Alu.mult, op1=Alu.mult)

                    # stage 1: gT[f,t] = silu(w1.T@snT) * (w2.T@snT)
                    gT = big.tile([128, 16, TT], f32r, name="gT", tag="gT")
                    for f in range(16):
                        fs = slice(128 * f, 128 * (f + 1))
                        pu1 = psU.tile([128, TT], f32, name="pu1", tag="u")
                        for c in range(4):
                            nc.tensor.matmul(pu1[:], w1_sb[:, c, fs], snT[:, c, :],
                                             start=(c == 0), stop=(c == 3))
                        pu2 = psU.tile([128, TT], f32, name="pu2", tag="u")
                        for c in range(4):
                            nc.tensor.matmul(pu2[:], w2_sb[:, c, fs], snT[:, c, :],
                                             start=(c == 0), stop=(c == 3))
                        sil = work.tile([128, TT], f32, name="sil", tag="sil")
                        nc.scalar.activation(sil[:], pu1[:], Act.Silu)
                        nc.vector.tensor_mul(gT[:, f, :], sil[:], pu2[:])

                    # stage 2: hT = w3.T@gT + seqT   (residual)
                    hT = big.tile([128, 4, TT], f32r, name="hT", tag="hT")
                    for m in range(4):
                        ms = slice(128 * m, 128 * (m + 1))
                        py = psY.tile([128, TT], f32, name="py", tag="yv")
                        for kf in range(16):
                            nc.tensor.matmul(py[:], w3_sb[:, kf, ms], gT[:, kf, :],
                                             start=(kf == 0), stop=(kf == 15))
                        nc.vector.tensor_add(hT[:, m, :], py[:], seqT[:, m, :])

                    # per-head value projection vT[e,t] (2 heads per psum tile)
                    vT = big.tile([128, 4, TT], f32, name="vT", tag="vT")
                    for c in range(4):
                        pv = psY.tile([128, TT], f32, name="pv", tag="yv")
                        nc.tensor.matmul(pv[:], wv_sb[:, c, :], hT[:, c, :],
                                         start=True, stop=True)
                        nc.scalar.copy(vT[:, c, :], pv[:])

                    # attention per batch row
                    for bb2 in range(2):
                        b = 2 * bi + bb2
                        off = bb2 * T
                        ps = psB.tile([NH, T], f32, name="ps", tag="sc")
                        for c in range(4):
                            tmp = work.tile([128, T], f32r, name="tmp", tag="tmp")
                            nc.vector.tensor_scalar_mul(tmp[:], hT[:, c, off:off + T],
                                                        qkT_sb[:, c, b:b + 1])
                            nc.tensor.matmul(ps[:], ones_blk[:, c, :], tmp[:],
                                             start=(c == 0), stop=False)
                        mp, mj = b // 16, b % 16
                        nc.tensor.matmul(ps[:], ones_m[64 * mp:64 * mp + 1, :],
                                         maskadd[64 * mp:64 * mp + 1, mj, :],
                                         start=False, stop=True)
                        negmx = attnp.tile([NH, 1], f32, name="negmx", tag="negmx")
                        nc.vector.tensor_reduce(negmx[:], ps[:], axis=mybir.AxisListType.X,
                                                op=Alu.max, negate=True)
                        attn_e = attnp.tile([NH, T], f32, name="attn_e", tag="attn_e")
                        den = attnp.tile([NH, 1], f32, name="den", tag="den")
                        nc.scalar.activation(attn_e[:], ps[:], Act.Exp,
                                             bias=negmx[:], scale=1.0, accum_out=den[:])
                        rden = attnp.tile([NH, 1], f32, name="rden", tag="rden")
                        nc.vector.reciprocal(rden[:], den[:])
                        attn_n = attnp.tile([NH, T], f32r, name="attn_n", tag="attn_n")
                        nc.vector.tensor_scalar_mul(attn_n[:], attn_e[:], rden[:])
                        for c in range(4):
                            pbc2 = psB.tile([128, T], f32, name="pbc2", tag="sc")
                            nc.tensor.matmul(pbc2[:], sel8[:, c, :], attn_n[:],
                                             start=True, stop=True)
                            scr = work.tile([128, T], f32, name="scr", tag="scr")
                            nc.vector.scalar_tensor_tensor(
                                scr[:], vT[:, c, off:off + T], 1.0, pbc2[:],
                                op0=Alu.mult, op1=Alu.mult,
                                accum_out=ctxT[:, c, b:b + 1])

                if repeat > 1:
                    nc.vector.tensor_add(dce_guard[:], dce_guard[:], ctxT[:])

            if repeat > 1:
                nc.vector.tensor_add(ctxT[:], ctxT[:], dce_guard[:])
                nc.vector.scalar_tensor_tensor(
                    ctxT[:], ctxT[:], 1.0 / (repeat + 1.0), dce_guard[:],
                    op0=Alu.mult, op1=Alu.bypass)

            # ---------------- output: transpose back + residual q ----------------
            ctx_tok = const.tile([BS, 4, 128], f32, name="ctx_tok")
            for c in range(4):
                pt = psA.tile([BS, 128], f32, name="pt", tag="ptrans")
                nc.tensor.transpose(pt[:], ctxT[:, c, :], ident[:])
                nc.vector.scalar_tensor_tensor(
                    ctx_tok[:, c, :], pt[:], 1.0, q_sb[:, 128 * c:128 * (c + 1)],
                    op0=Alu.mult, op1=Alu.add)
            nc.sync.dma_start(out_d.ap(), ctx_tok[:].rearrange("b c p -> b (c p)"))

    nc.compile()
    return nc


def _make_runner(nc, n_cores=NCORES):
    import jax
    from jax.sharding import Mesh, PartitionSpec, NamedSharding
    from jax.experimental.shard_map import shard_map
    from concourse import mybir as _mybir
    from concourse import bass2jax

    bass2jax.install_neuronx_cc_hook()
    partition_name = nc.partition_id_tensor.name if nc.partition_id_tensor else None
    in_names, out_names, out_avals, zero_outs = [], [], [], []
    for alloc in nc.m.functions[0].allocations:
        if not isinstance(alloc, _mybir.MemoryLocationSet):
            continue
        name = alloc.memorylocations[0].name
        if alloc.kind == "ExternalInput":
            if name != partition_name:
                in_names.append(name)
        elif alloc.kind == "ExternalOutput":
            out_names.append(name)
            shape = tuple(alloc.tensor_shape)
            dtype = _mybir.dt.np(alloc.dtype)
            out_avals.append(jax.core.ShapedArray(shape, dtype))
            zero_outs.append(np.zeros(shape, dtype))
    n_params = len(in_names)
    all_in_names = list(in_names) + list(out_names)
    if partition_name is not None:
        all_in_names.append(partition_name)

    def _body(*args):
        operands = list(args)
        if partition_name is not None:
            operands.append(bass2jax.partition_id_tensor())
        outs = bass2jax._bass_exec_p.bind(
            *operands,
            out_avals=tuple(out_avals),
            in_names=tuple(all_in_names),
            out_names=tuple(out_names),
            lowering_input_output_aliases=(),
            sim_require_finite=True,
            sim_require_nnan=True,
            nc=nc,
        )
        return tuple(outs)

    devices = jax.devices()[:n_cores]
    mesh = Mesh(np.asarray(devices), ("core",))
    n_outs = len(out_avals)
    in_specs = (PartitionSpec("core"),) * (n_params + n_outs)
    out_specs = (PartitionSpec("core"),) * len(out_names)
    sharded = jax.jit(
        shard_map(_body, mesh=mesh, in_specs=in_specs, out_specs=out_specs,
                  check_rep=False),
        keep_unused=True)
    sharding = NamedSharding(mesh, PartitionSpec("core"))

    def run(in_maps, timing_iters=0):
        import time
        per_core = [[np.asarray(m[name]) for name in in_names] for m in in_maps]
        concat_in = [np.concatenate([per_core[c][i] for c in range(n_cores)], axis=0)
                     for i in range(n_params)]
        concat_zeros = [np.zeros((n_cores * z.shape[0], *z.shape[1:]), z.dtype)
                        for z in zero_outs]
        dev_in = [jax.device_put(x, sharding) for x in concat_in]
        dev_zero = [jax.device_put(x, sharding) for x in concat_zeros]
        out = sharded(*dev_in, *dev_zero)
        jax.block_until_ready(out)
        dt = None
        if timing_iters:
            t0 = time.perf_counter()
            for _ in range(timing_iters):
                out = sharded(*dev_in, *dev_zero)
            jax.block_until_ready(out)
            dt = (time.perf_counter() - t0) / timing_iters
        results = [
            {name: np.asarray(out[i]).reshape(n_cores, *out_avals[i].shape)[c]
             for i, name in enumerate(out_names)}
            for c in range(n_cores)
        ]
        return results, dt

    return run


def _get_runner(repeat=1):
    if repeat not in _RUNNERS:
        nc = _build_program(repeat=repeat)
        _RUNNERS[repeat] = _make_runner(nc)
    return _RUNNERS[repeat]


def _in_maps(q, seq, rms_w, w1, w2, w3, w_k, w_v, seq_mask):
    q = np.asarray(q, np.float32).reshape(B, MODEL)
    seq = np.asarray(seq, np.float32)
    mask = np.asarray(seq_mask).astype(np.uint8)
    rms_w = np.asarray(rms_w, np.float32)
    w1 = np.asarray(w1, np.float32)
    w2 = np.asarray(w2, np.float32)
    w3 = np.asarray(w3, np.float32)
    w_k = np.asarray(w_k, np.float32)
    w_v = np.asarray(w_v, np.float32)
    maps = []
    for c in range(NCORES):
        s = slice(c * BS, (c + 1) * BS)
        maps.append({
            "seq": seq[s], "q": q[s], "mask": mask[s], "rms_w": rms_w,
            "w1": w1, "w2": w2, "w3": w3, "w_k": w_k, "w_v": w_v,
        })
    return maps


def kernel(q, seq, rms_w, w1, w2, w3, w_k, w_v, seq_mask, _timing_iters=0, _repeat=1):
    run = _get_runner(_repeat)
    maps = _in_maps(q, seq, rms_w, w1, w2, w3, w_k, w_v, seq_mask)
    results, dt = run(maps, timing_iters=_timing_iters)
    out = np.concatenate([r["out"] for r in results], axis=0).reshape(B, NH, D)
    if _timing_iters:
        return out, dt
    return out
